# revision 37
# baseline (speedup 1.0000x reference)
"""BiquadCell Trainium2 kernel.

Reference semantics (per batch lane b):
    o_t = tanh(w0*x0 + w1*x1 + (w2+1)*x2 + w3*o_{t-1} + w4*o_{t-2})
with (o_{-1}, o_{-2}) = carry[b].

Strategy:
  - Shard batch B=2048 across 8 cores (256 lanes each).
  - The recurrence is contractive (companion spectral radius ~0.2 for the
    given weights), so initial-state influence decays geometrically.  Split
    T=16384 into 128 chunks of C=128 steps; each chunk starts from a zero
    state guess and runs W=16 warmup steps first -- after warmup its state
    matches the true scan to far below fp32 resolution.  All 128 chunks
    advance in lockstep: chunk = SBUF partition, lane = free dim, so every
    scan step is a [128, 256] instruction instead of a [*, tiny] one.
  - Chunk 0 has no predecessor: its warmup input is zeroed (state stays 0)
    and its true initial state is patched in from `carry` at t=0/t=1 via
    partition-0-only instructions.
  - Everything is expressed in a z/w1-scaled basis so the input projection
    is 2 fused mult-add ops and the ACT instruction's free `scale` restores
    the w1 factor inside tanh:
        h   = x0*(w0/w2p) + x2          (w2p = w2+1)
        z'  = h*(w2p/w1) + x1           == z/w1
        u'  = o_{t-2}*(w4/w1) + z'
        v'  = o_{t-1}*(w3/w1) + u'
        o_t = tanh(w1 * v')
    Work is split across DVE / Pool(GpSimd) / ACT to balance engine time.
"""

import numpy as np

T = 16384
B = 2048
NCORES = 8
L = B // NCORES          # 256 lanes per core
C = 128                  # chunk length
W = 16                   # warmup steps
NCH = T // C             # 128 chunks == SBUF partitions
S = C + W                # scan steps
SB = 8                   # steps per streamed block
NB = S // SB             # blocks
FR = L * 3               # floats per x row (per core)

_cache = {}


def _build(w):
    import concourse.bass as bass
    import concourse.bacc as bacc
    import concourse.tile as tile
    import concourse.mybir as mybir

    w0, w1, w2, w3, w4 = [float(v) for v in np.asarray(w, np.float32).reshape(-1)]
    w2p = w2 + 1.0
    f32 = mybir.dt.float32
    AF = mybir.ActivationFunctionType
    OP = mybir.AluOpType

    # scaled-basis constants (fall back to unscaled if w1/w2p are degenerate)
    scaled = abs(w1) > 1e-3 and abs(w2p) > 1e-3
    if scaled:
        k_h = w0 / w2p          # h  = x0*k_h + x2
        k_z = w2p / w1          # z' = h*k_z + x1
        k_u = w4 / w1           # u' = o2*k_u + z'
        k_v = w3 / w1           # v' = o1*k_v + u'
        sc_act = w1             # o = tanh(sc_act * v')
    else:
        k_u, k_v, sc_act = w4, w3, 1.0

    nc = bacc.Bacc("TRN2", target_bir_lowering=False, debug=False, num_devices=NCORES)
    x = nc.dram_tensor("inputs", [T, L, 3], f32, kind="ExternalInput")
    cr = nc.dram_tensor("carry", [L, 2], f32, kind="ExternalInput")
    out = nc.dram_tensor("out", [T, L], f32, kind="ExternalOutput")

    with tile.TileContext(nc) as tc:
        with tc.tile_pool(name="xp", bufs=4) as xp, \
             tc.tile_pool(name="zp", bufs=4) as zp, \
             tc.tile_pool(name="tp", bufs=2) as tp, \
             tc.tile_pool(name="op", bufs=4) as opool, \
             tc.tile_pool(name="sp", bufs=4) as sp, \
             tc.tile_pool(name="cp", bufs=1) as cp:
            # carry -> [1, 512] tile; strided views give the two columns
            cin = cp.tile([1, 2 * L], f32, tag="cin")
            nc.sync.dma_start(out=cin[:], in_=bass.AP(cr, 0, [[2 * L, 1], [1, 2 * L]]))
            c_r = cin[:].rearrange("p (n c) -> p n c", c=2)
            c0 = c_r[:, :, 0:1]   # [1, 256, 1] o_{t-1} init for chunk 0
            c1 = c_r[:, :, 1:2]   # [1, 256, 1] o_{t-2} init for chunk 0

            zinit = cp.tile([128, 2 * L], f32, tag="zinit")
            nc.gpsimd.memset(zinit[:], 0.0)
            # rolling full-width refs to o_{t-1} / o_{t-2} (halves are slices)
            o1 = zinit[:, 0:L]
            o2 = zinit[:, L:2 * L]

            def dma_block(k, pieces=1):
                s0 = k * SB
                warm = (s0 + SB) <= W   # block entirely inside warmup
                xt = xp.tile([128, SB * FR], f32, tag="x")
                xt3 = xt[:].rearrange("p (n c) -> p n c", c=FR)
                rp = SB // pieces
                for i in range(pieces):
                    if warm:
                        # partition 0 (chunk 0) has no t<0 data and is left
                        # uninitialized: its warmup values are garbage but the
                        # gs==W / gs==W+1 carry patches fully overwrite its
                        # state before any of its outputs are stored
                        off = (s0 - W + C + i * rp) * FR
                        nc.sync.dma_start(
                            out=xt3[1:128, i * rp:(i + 1) * rp],
                            in_=bass.AP(x, off, [[C * FR, 127], [FR, rp], [1, FR]]))
                    else:
                        off = (s0 - W + i * rp) * FR
                        nc.sync.dma_start(
                            out=xt3[:, i * rp:(i + 1) * rp],
                            in_=bass.AP(x, off, [[C * FR, 128], [FR, rp], [1, FR]]))
                return xt

            HL = L // 2  # 128 lanes per half-chain

            def proj_sliver(k, xt, z, s, nsteps=1):
                # nsteps steps' worth of block k's projection, emitted inside
                # the previous block's scan so it fills engine idle windows
                # instead of stalling the recurrence chain
                xr = xt[:].rearrange("p (n c) -> p n c", c=3)
                lo, hi = s * L, (s + nsteps) * L
                x0 = xr[:, lo:hi, 0:1]
                x1 = xr[:, lo:hi, 1:2]
                x2 = xr[:, lo:hi, 2:3]
                zv = z[:].rearrange("p (n c) -> p n c", c=1)[:, lo:hi, :]
                h = sp.tile([128, 4 * L], f32, tag="hs")
                h3 = h[:].rearrange("p (n c) -> p n c", c=1)[:, 0:(hi - lo), :]
                if scaled:
                    # Pool does the 2-op h build (ts+tt legal there); DVE only
                    # pays one fused op for z'
                    nc.gpsimd.tensor_scalar_mul(h3[:], x0, k_h)
                    nc.gpsimd.tensor_add(h3[:], h3[:], x2)
                    nc.vector.scalar_tensor_tensor(zv, h3[:], k_z, x1, op0=OP.mult, op1=OP.add)
                else:
                    nc.gpsimd.tensor_scalar_mul(h3[:], x0, w0)
                    b = sp.tile([128, 4 * L], f32, tag="bs")
                    b3 = b[:].rearrange("p (n c) -> p n c", c=1)[:, 0:(hi - lo), :]
                    nc.vector.scalar_tensor_tensor(b3[:], x1, w1, h3[:], op0=OP.mult, op1=OP.add)
                    nc.vector.scalar_tensor_tensor(zv, x2, w2p, b3[:], op0=OP.mult, op1=OP.add)

            # software pipeline: x-DMA runs 2 blocks ahead; block k+1's
            # projection is emitted sliver-by-sliver during block k's scan
            xts = {0: dma_block(0, pieces=4), 1: dma_block(1)}
            z0 = zp.tile([128, SB * L], f32, tag="z")
            for s in range(0, SB, 2):
                proj_sliver(0, xts[0], z0, s, nsteps=2)
            zs_blocks = {0: z0}

            pending_out = []

            def fix_p0(eng, dst, cinit, src, kk):
                # overwrite partition 0 (chunk 0) with the carry-based value
                eng.scalar_tensor_tensor(
                    dst[0:1].rearrange("p (n c) -> p n c", c=1), cinit, kk,
                    src[0:1].rearrange("p (n c) -> p n c", c=1), op0=OP.mult, op1=OP.add)

            for k in range(NB):
                s0 = k * SB
                warm = (s0 + SB) <= W
                if k + 2 < NB:
                    xts[k + 2] = dma_block(k + 2)
                if k + 1 < NB:
                    znext = zp.tile([128, SB * L], f32, tag="z")
                    zs_blocks[k + 1] = znext
                z = zs_blocks.pop(k)

                ob = opool.tile([128, SB * L], f32, tag="ob")
                for s in range(SB):
                    gs = s0 + s
                    lo = s * L
                    zs = z[:, lo:lo + L]
                    o1A, o1B = o1[:, 0:HL], o1[:, HL:L]
                    u = sp.tile([128, L], f32, tag="u")
                    v = sp.tile([128, L], f32, tag="v")
                    vA, vB = v[:, 0:HL], v[:, HL:L]
                    # u is 2 steps off the critical path; emitted before the
                    # v's so it fills DVE's wait-for-tanh windows
                    nc.vector.scalar_tensor_tensor(u[:], o2, k_u, zs, op0=OP.mult, op1=OP.add)
                    if gs == W:  # chunk 0, t=0: o_{t-2} is carry col 1
                        fix_p0(nc.vector, u, c1, zs, k_u)
                    elif gs == W + 1:  # chunk 0, t=1: o_{t-2} is carry col 0
                        fix_p0(nc.vector, u, c0, zs, k_u)
                    # two half-lane chains: B's tanh overlaps A's handoff
                    nc.vector.scalar_tensor_tensor(vA, o1A, k_v, u[:, 0:HL], op0=OP.mult, op1=OP.add)
                    nc.vector.scalar_tensor_tensor(vB, o1B, k_v, u[:, HL:L], op0=OP.mult, op1=OP.add)
                    if gs == W:  # chunk 0, t=0: o_{t-1} is carry col 0
                        fix_p0(nc.vector, v, c0, u, k_v)
                    oslotA = ob[:, lo:lo + HL]
                    oslotB = ob[:, lo + HL:lo + L]
                    nc.scalar.activation(oslotA[:], vA[:], AF.Tanh, bias=0.0, scale=sc_act)
                    nc.scalar.activation(oslotB[:], vB[:], AF.Tanh, bias=0.0, scale=sc_act)
                    if k + 1 < NB and s % 4 == 0:
                        proj_sliver(k + 1, xts[k + 1], zs_blocks[k + 1], s, nsteps=4)
                    o2 = o1
                    o1 = ob[:, lo:lo + L]
                if not warm:
                    pending_out.append((ob, s0))
                # delay out-DMA issue ~2 blocks so the transfers queue behind
                # later x-stream reads: the input stream finishes sooner and
                # never throttles the recurrence chain (issued from ACT so the
                # wait-on-tanh can't head-of-line-block SP's x-DMAs either)
                if len(pending_out) > 2:
                    dob, ds0 = pending_out.pop(0)
                    dob3 = dob[:].rearrange("p (s l) -> p s l", l=L)
                    nc.scalar.dma_start(
                        out=bass.AP(out, (ds0 - W) * L, [[C * L, 128], [L, SB], [1, L]]),
                        in_=dob3[:])
            while pending_out:
                dob, ds0 = pending_out.pop(0)
                dob3 = dob[:].rearrange("p (s l) -> p s l", l=L)
                nc.scalar.dma_start(
                    out=bass.AP(out, (ds0 - W) * L, [[C * L, 128], [L, SB], [1, L]]),
                    in_=dob3[:])
    nc.compile()
    return nc


def kernel(inputs, carry, weights):
    from concourse.bass_utils import run_bass_kernel_spmd

    key = np.asarray(weights, np.float32).tobytes()
    if key not in _cache:
        _cache[key] = _build(weights)
    nc = _cache[key]

    x = np.ascontiguousarray(np.asarray(inputs, np.float32))
    cr = np.ascontiguousarray(np.asarray(carry, np.float32))
    in_maps = []
    for c in range(NCORES):
        sl = slice(c * L, (c + 1) * L)
        in_maps.append({
            "inputs": np.ascontiguousarray(x[:, sl, :]),
            "carry": np.ascontiguousarray(cr[sl, :]),
        })
    res = run_bass_kernel_spmd(nc, in_maps, core_ids=list(range(NCORES)))
    outs = [r["out"] for r in res.results]
    return np.concatenate([o[:, :, None] for o in outs], axis=1)


# revision 38
# speedup vs baseline: 1.0193x; 1.0193x over previous
"""BiquadCell Trainium2 kernel.

Reference semantics (per batch lane b):
    o_t = tanh(w0*x0 + w1*x1 + (w2+1)*x2 + w3*o_{t-1} + w4*o_{t-2})
with (o_{-1}, o_{-2}) = carry[b].

Strategy:
  - Shard batch B=2048 across 8 cores (256 lanes each).
  - The recurrence is contractive (companion spectral radius ~0.2 for the
    given weights), so initial-state influence decays geometrically.  Split
    T=16384 into 128 chunks of C=128 steps; each chunk starts from a zero
    state guess and runs W=16 warmup steps first -- after warmup its state
    matches the true scan to far below fp32 resolution.  All 128 chunks
    advance in lockstep: chunk = SBUF partition, lane = free dim, so every
    scan step is a [128, 256] instruction instead of a [*, tiny] one.
  - Chunk 0 has no predecessor: its warmup input is zeroed (state stays 0)
    and its true initial state is patched in from `carry` at t=0/t=1 via
    partition-0-only instructions.
  - Everything is expressed in a z/w1-scaled basis so the input projection
    is 2 fused mult-add ops and the ACT instruction's free `scale` restores
    the w1 factor inside tanh:
        h   = x0*(w0/w2p) + x2          (w2p = w2+1)
        z'  = h*(w2p/w1) + x1           == z/w1
        u'  = o_{t-2}*(w4/w1) + z'
        v'  = o_{t-1}*(w3/w1) + u'
        o_t = tanh(w1 * v')
    Work is split across DVE / Pool(GpSimd) / ACT to balance engine time.
"""

import numpy as np

T = 16384
B = 2048
NCORES = 8
L = B // NCORES          # 256 lanes per core
C = 128                  # chunk length
W = 16                   # warmup steps
NCH = T // C             # 128 chunks == SBUF partitions
S = C + W                # scan steps
SB = 4                   # steps per streamed block
NB = S // SB             # blocks
FR = L * 3               # floats per x row (per core)

_cache = {}


def _build(w):
    import concourse.bass as bass
    import concourse.bacc as bacc
    import concourse.tile as tile
    import concourse.mybir as mybir

    w0, w1, w2, w3, w4 = [float(v) for v in np.asarray(w, np.float32).reshape(-1)]
    w2p = w2 + 1.0
    f32 = mybir.dt.float32
    AF = mybir.ActivationFunctionType
    OP = mybir.AluOpType

    # scaled-basis constants (fall back to unscaled if w1/w2p are degenerate)
    scaled = abs(w1) > 1e-3 and abs(w2p) > 1e-3
    if scaled:
        k_h = w0 / w2p          # h  = x0*k_h + x2
        k_z = w2p / w1          # z' = h*k_z + x1
        k_u = w4 / w1           # u' = o2*k_u + z'
        k_v = w3 / w1           # v' = o1*k_v + u'
        sc_act = w1             # o = tanh(sc_act * v')
    else:
        k_u, k_v, sc_act = w4, w3, 1.0

    nc = bacc.Bacc("TRN2", target_bir_lowering=False, debug=False, num_devices=NCORES)
    x = nc.dram_tensor("inputs", [T, L, 3], f32, kind="ExternalInput")
    cr = nc.dram_tensor("carry", [L, 2], f32, kind="ExternalInput")
    out = nc.dram_tensor("out", [T, L], f32, kind="ExternalOutput")

    with tile.TileContext(nc) as tc:
        with tc.tile_pool(name="xp", bufs=6) as xp, \
             tc.tile_pool(name="zp", bufs=8) as zp, \
             tc.tile_pool(name="tp", bufs=2) as tp, \
             tc.tile_pool(name="op", bufs=8) as opool, \
             tc.tile_pool(name="sp", bufs=4) as sp, \
             tc.tile_pool(name="cp", bufs=1) as cp:
            # carry -> [1, 512] tile; strided views give the two columns
            cin = cp.tile([1, 2 * L], f32, tag="cin")
            nc.sync.dma_start(out=cin[:], in_=bass.AP(cr, 0, [[2 * L, 1], [1, 2 * L]]))
            c_r = cin[:].rearrange("p (n c) -> p n c", c=2)
            c0 = c_r[:, :, 0:1]   # [1, 256, 1] o_{t-1} init for chunk 0
            c1 = c_r[:, :, 1:2]   # [1, 256, 1] o_{t-2} init for chunk 0

            zinit = cp.tile([128, 2 * L], f32, tag="zinit")
            nc.gpsimd.memset(zinit[:], 0.0)
            # rolling full-width refs to o_{t-1} / o_{t-2} (halves are slices)
            o1 = zinit[:, 0:L]
            o2 = zinit[:, L:2 * L]

            def dma_block(k, pieces=1):
                s0 = k * SB
                warm = (s0 + SB) <= W   # block entirely inside warmup
                xt = xp.tile([128, SB * FR], f32, tag="x")
                xt3 = xt[:].rearrange("p (n c) -> p n c", c=FR)
                rp = SB // pieces
                for i in range(pieces):
                    if warm:
                        # partition 0 (chunk 0) has no t<0 data and is left
                        # uninitialized: its warmup values are garbage but the
                        # gs==W / gs==W+1 carry patches fully overwrite its
                        # state before any of its outputs are stored
                        off = (s0 - W + C + i * rp) * FR
                        nc.sync.dma_start(
                            out=xt3[1:128, i * rp:(i + 1) * rp],
                            in_=bass.AP(x, off, [[C * FR, 127], [FR, rp], [1, FR]]))
                    else:
                        off = (s0 - W + i * rp) * FR
                        nc.sync.dma_start(
                            out=xt3[:, i * rp:(i + 1) * rp],
                            in_=bass.AP(x, off, [[C * FR, 128], [FR, rp], [1, FR]]))
                return xt

            HL = L // 2  # 128 lanes per half-chain

            def proj_sliver(k, xt, z, s, nsteps=1):
                # nsteps steps' worth of block k's projection, emitted inside
                # the previous block's scan so it fills engine idle windows
                # instead of stalling the recurrence chain
                xr = xt[:].rearrange("p (n c) -> p n c", c=3)
                lo, hi = s * L, (s + nsteps) * L
                x0 = xr[:, lo:hi, 0:1]
                x1 = xr[:, lo:hi, 1:2]
                x2 = xr[:, lo:hi, 2:3]
                zv = z[:].rearrange("p (n c) -> p n c", c=1)[:, lo:hi, :]
                h = sp.tile([128, 4 * L], f32, tag="hs")
                h3 = h[:].rearrange("p (n c) -> p n c", c=1)[:, 0:(hi - lo), :]
                if scaled:
                    # Pool does the 2-op h build (ts+tt legal there); DVE only
                    # pays one fused op for z'
                    nc.gpsimd.tensor_scalar_mul(h3[:], x0, k_h)
                    nc.gpsimd.tensor_add(h3[:], h3[:], x2)
                    nc.vector.scalar_tensor_tensor(zv, h3[:], k_z, x1, op0=OP.mult, op1=OP.add)
                else:
                    nc.gpsimd.tensor_scalar_mul(h3[:], x0, w0)
                    b = sp.tile([128, 4 * L], f32, tag="bs")
                    b3 = b[:].rearrange("p (n c) -> p n c", c=1)[:, 0:(hi - lo), :]
                    nc.vector.scalar_tensor_tensor(b3[:], x1, w1, h3[:], op0=OP.mult, op1=OP.add)
                    nc.vector.scalar_tensor_tensor(zv, x2, w2p, b3[:], op0=OP.mult, op1=OP.add)

            # software pipeline: x-DMA runs 2 blocks ahead; block k+1's
            # projection is emitted sliver-by-sliver during block k's scan
            xts = {0: dma_block(0, pieces=2), 1: dma_block(1)}
            z0 = zp.tile([128, SB * L], f32, tag="z")
            for s in range(0, SB, 2):
                proj_sliver(0, xts[0], z0, s, nsteps=2)
            zs_blocks = {0: z0}

            pending_out = []

            def fix_p0(eng, dst, cinit, src, kk):
                # overwrite partition 0 (chunk 0) with the carry-based value
                eng.scalar_tensor_tensor(
                    dst[0:1].rearrange("p (n c) -> p n c", c=1), cinit, kk,
                    src[0:1].rearrange("p (n c) -> p n c", c=1), op0=OP.mult, op1=OP.add)

            for k in range(NB):
                s0 = k * SB
                warm = (s0 + SB) <= W
                if k + 2 < NB:
                    xts[k + 2] = dma_block(k + 2)
                if k + 1 < NB:
                    znext = zp.tile([128, SB * L], f32, tag="z")
                    zs_blocks[k + 1] = znext
                z = zs_blocks.pop(k)

                ob = opool.tile([128, SB * L], f32, tag="ob")
                for s in range(SB):
                    gs = s0 + s
                    lo = s * L
                    zs = z[:, lo:lo + L]
                    o1A, o1B = o1[:, 0:HL], o1[:, HL:L]
                    u = sp.tile([128, L], f32, tag="u")
                    v = sp.tile([128, L], f32, tag="v")
                    vA, vB = v[:, 0:HL], v[:, HL:L]
                    # u is 2 steps off the critical path; emitted before the
                    # v's so it fills DVE's wait-for-tanh windows
                    nc.vector.scalar_tensor_tensor(u[:], o2, k_u, zs, op0=OP.mult, op1=OP.add)
                    if gs == W:  # chunk 0, t=0: o_{t-2} is carry col 1
                        fix_p0(nc.vector, u, c1, zs, k_u)
                    elif gs == W + 1:  # chunk 0, t=1: o_{t-2} is carry col 0
                        fix_p0(nc.vector, u, c0, zs, k_u)
                    # two half-lane chains: B's tanh overlaps A's handoff
                    nc.vector.scalar_tensor_tensor(vA, o1A, k_v, u[:, 0:HL], op0=OP.mult, op1=OP.add)
                    nc.vector.scalar_tensor_tensor(vB, o1B, k_v, u[:, HL:L], op0=OP.mult, op1=OP.add)
                    if gs == W:  # chunk 0, t=0: o_{t-1} is carry col 0
                        fix_p0(nc.vector, v, c0, u, k_v)
                    oslotA = ob[:, lo:lo + HL]
                    oslotB = ob[:, lo + HL:lo + L]
                    nc.scalar.activation(oslotA[:], vA[:], AF.Tanh, bias=0.0, scale=sc_act)
                    nc.scalar.activation(oslotB[:], vB[:], AF.Tanh, bias=0.0, scale=sc_act)
                    if k + 1 < NB and s % 4 == 0:
                        proj_sliver(k + 1, xts[k + 1], zs_blocks[k + 1], s, nsteps=4)
                    o2 = o1
                    o1 = ob[:, lo:lo + L]
                if not warm:
                    pending_out.append((ob, s0))
                # delay out-DMA issue ~2 blocks so the transfers queue behind
                # later x-stream reads: the input stream finishes sooner and
                # never throttles the recurrence chain (issued from ACT so the
                # wait-on-tanh can't head-of-line-block SP's x-DMAs either)
                if len(pending_out) > 2:
                    dob, ds0 = pending_out.pop(0)
                    dob3 = dob[:].rearrange("p (s l) -> p s l", l=L)
                    nc.scalar.dma_start(
                        out=bass.AP(out, (ds0 - W) * L, [[C * L, 128], [L, SB], [1, L]]),
                        in_=dob3[:])
            while pending_out:
                dob, ds0 = pending_out.pop(0)
                dob3 = dob[:].rearrange("p (s l) -> p s l", l=L)
                nc.scalar.dma_start(
                    out=bass.AP(out, (ds0 - W) * L, [[C * L, 128], [L, SB], [1, L]]),
                    in_=dob3[:])
    nc.compile()
    return nc


def kernel(inputs, carry, weights):
    from concourse.bass_utils import run_bass_kernel_spmd

    key = np.asarray(weights, np.float32).tobytes()
    if key not in _cache:
        _cache[key] = _build(weights)
    nc = _cache[key]

    x = np.ascontiguousarray(np.asarray(inputs, np.float32))
    cr = np.ascontiguousarray(np.asarray(carry, np.float32))
    in_maps = []
    for c in range(NCORES):
        sl = slice(c * L, (c + 1) * L)
        in_maps.append({
            "inputs": np.ascontiguousarray(x[:, sl, :]),
            "carry": np.ascontiguousarray(cr[sl, :]),
        })
    res = run_bass_kernel_spmd(nc, in_maps, core_ids=list(range(NCORES)))
    outs = [r["out"] for r in res.results]
    return np.concatenate([o[:, :, None] for o in outs], axis=1)


# revision 39
# speedup vs baseline: 1.0517x; 1.0318x over previous
"""BiquadCell Trainium2 kernel.

Reference semantics (per batch lane b):
    o_t = tanh(w0*x0 + w1*x1 + (w2+1)*x2 + w3*o_{t-1} + w4*o_{t-2})
with (o_{-1}, o_{-2}) = carry[b].

Strategy:
  - Shard batch B=2048 across 8 cores (256 lanes each).
  - The recurrence is contractive (companion spectral radius ~0.2 for the
    given weights), so initial-state influence decays geometrically.  Split
    T=16384 into 128 chunks of C=128 steps; each chunk starts from a zero
    state guess and runs W=16 warmup steps first -- after warmup its state
    matches the true scan to far below fp32 resolution.  All 128 chunks
    advance in lockstep: chunk = SBUF partition, lane = free dim, so every
    scan step is a [128, 256] instruction instead of a [*, tiny] one.
  - Chunk 0 has no predecessor: its warmup input is zeroed (state stays 0)
    and its true initial state is patched in from `carry` at t=0/t=1 via
    partition-0-only instructions.
  - Everything is expressed in a z/w1-scaled basis so the input projection
    is 2 fused mult-add ops and the ACT instruction's free `scale` restores
    the w1 factor inside tanh:
        h   = x0*(w0/w2p) + x2          (w2p = w2+1)
        z'  = h*(w2p/w1) + x1           == z/w1
        u'  = o_{t-2}*(w4/w1) + z'
        v'  = o_{t-1}*(w3/w1) + u'
        o_t = tanh(w1 * v')
    Work is split across DVE / Pool(GpSimd) / ACT to balance engine time.
"""

import numpy as np

T = 16384
B = 2048
NCORES = 8
L = B // NCORES          # 256 lanes per core
C = 128                  # chunk length
W = 16                   # warmup steps
NCH = T // C             # 128 chunks == SBUF partitions
S = C + W                # scan steps
SB = 4                   # steps per streamed block
NB = S // SB             # blocks
FR = L * 3               # floats per x row (per core)

_cache = {}


def _build(w):
    import concourse.bass as bass
    import concourse.bacc as bacc
    import concourse.tile as tile
    import concourse.mybir as mybir

    w0, w1, w2, w3, w4 = [float(v) for v in np.asarray(w, np.float32).reshape(-1)]
    w2p = w2 + 1.0
    f32 = mybir.dt.float32
    AF = mybir.ActivationFunctionType
    OP = mybir.AluOpType

    # scaled-basis constants (fall back to unscaled if w1/w2p are degenerate)
    scaled = abs(w1) > 1e-3 and abs(w2p) > 1e-3
    if scaled:
        k_h = w0 / w2p          # h  = x0*k_h + x2
        k_z = w2p / w1          # z' = h*k_z + x1
        k_u = w4 / w1           # u' = o2*k_u + z'
        k_v = w3 / w1           # v' = o1*k_v + u'
        sc_act = w1             # o = tanh(sc_act * v')
    else:
        k_u, k_v, sc_act = w4, w3, 1.0

    nc = bacc.Bacc("TRN2", target_bir_lowering=False, debug=False, num_devices=NCORES)
    x = nc.dram_tensor("inputs", [T, L, 3], f32, kind="ExternalInput")
    cr = nc.dram_tensor("carry", [L, 2], f32, kind="ExternalInput")
    out = nc.dram_tensor("out", [T, L], f32, kind="ExternalOutput")

    with tile.TileContext(nc) as tc:
        with tc.tile_pool(name="xp", bufs=6) as xp, \
             tc.tile_pool(name="zp", bufs=8) as zp, \
             tc.tile_pool(name="tp", bufs=2) as tp, \
             tc.tile_pool(name="op", bufs=10) as opool, \
             tc.tile_pool(name="sp", bufs=4) as sp, \
             tc.tile_pool(name="cp", bufs=1) as cp:
            # carry -> [1, 512] tile; strided views give the two columns
            cin = cp.tile([1, 2 * L], f32, tag="cin")
            nc.sync.dma_start(out=cin[:], in_=bass.AP(cr, 0, [[2 * L, 1], [1, 2 * L]]))
            c_r = cin[:].rearrange("p (n c) -> p n c", c=2)
            c0 = c_r[:, :, 0:1]   # [1, 256, 1] o_{t-1} init for chunk 0
            c1 = c_r[:, :, 1:2]   # [1, 256, 1] o_{t-2} init for chunk 0

            zinit = cp.tile([128, 2 * L], f32, tag="zinit")
            nc.gpsimd.memset(zinit[:], 0.0)
            # rolling full-width refs to o_{t-1} / o_{t-2} (halves are slices)
            o1 = zinit[:, 0:L]
            o2 = zinit[:, L:2 * L]

            def dma_block(k, pieces=1):
                s0 = k * SB
                warm = (s0 + SB) <= W   # block entirely inside warmup
                xt = xp.tile([128, SB * FR], f32, tag="x")
                xt3 = xt[:].rearrange("p (n c) -> p n c", c=FR)
                rp = SB // pieces
                for i in range(pieces):
                    if warm:
                        # partition 0 (chunk 0) has no t<0 data and is left
                        # uninitialized: its warmup values are garbage but the
                        # gs==W / gs==W+1 carry patches fully overwrite its
                        # state before any of its outputs are stored
                        off = (s0 - W + C + i * rp) * FR
                        nc.sync.dma_start(
                            out=xt3[1:128, i * rp:(i + 1) * rp],
                            in_=bass.AP(x, off, [[C * FR, 127], [FR, rp], [1, FR]]))
                    else:
                        off = (s0 - W + i * rp) * FR
                        nc.sync.dma_start(
                            out=xt3[:, i * rp:(i + 1) * rp],
                            in_=bass.AP(x, off, [[C * FR, 128], [FR, rp], [1, FR]]))
                return xt

            HL = L // 2  # 128 lanes per half-chain

            def proj_sliver(k, xt, z, s, nsteps=1):
                # nsteps steps' worth of block k's projection, emitted inside
                # the previous block's scan so it fills engine idle windows
                # instead of stalling the recurrence chain
                xr = xt[:].rearrange("p (n c) -> p n c", c=3)
                lo, hi = s * L, (s + nsteps) * L
                x0 = xr[:, lo:hi, 0:1]
                x1 = xr[:, lo:hi, 1:2]
                x2 = xr[:, lo:hi, 2:3]
                zv = z[:].rearrange("p (n c) -> p n c", c=1)[:, lo:hi, :]
                h = sp.tile([128, 4 * L], f32, tag="hs")
                h3 = h[:].rearrange("p (n c) -> p n c", c=1)[:, 0:(hi - lo), :]
                if scaled:
                    # Pool does the 2-op h build (ts+tt legal there); DVE only
                    # pays one fused op for z'
                    nc.gpsimd.tensor_scalar_mul(h3[:], x0, k_h)
                    nc.gpsimd.tensor_add(h3[:], h3[:], x2)
                    nc.vector.scalar_tensor_tensor(zv, h3[:], k_z, x1, op0=OP.mult, op1=OP.add)
                else:
                    nc.gpsimd.tensor_scalar_mul(h3[:], x0, w0)
                    b = sp.tile([128, 4 * L], f32, tag="bs")
                    b3 = b[:].rearrange("p (n c) -> p n c", c=1)[:, 0:(hi - lo), :]
                    nc.vector.scalar_tensor_tensor(b3[:], x1, w1, h3[:], op0=OP.mult, op1=OP.add)
                    nc.vector.scalar_tensor_tensor(zv, x2, w2p, b3[:], op0=OP.mult, op1=OP.add)

            # software pipeline: x-DMA runs 2 blocks ahead; block k+1's
            # projection is emitted sliver-by-sliver during block k's scan
            xts = {0: dma_block(0, pieces=2), 1: dma_block(1)}
            z0 = zp.tile([128, SB * L], f32, tag="z")
            for s in range(0, SB, 2):
                proj_sliver(0, xts[0], z0, s, nsteps=2)
            zs_blocks = {0: z0}

            pending_out = []

            def fix_p0(eng, dst, cinit, src, kk):
                # overwrite partition 0 (chunk 0) with the carry-based value
                eng.scalar_tensor_tensor(
                    dst[0:1].rearrange("p (n c) -> p n c", c=1), cinit, kk,
                    src[0:1].rearrange("p (n c) -> p n c", c=1), op0=OP.mult, op1=OP.add)

            for k in range(NB):
                s0 = k * SB
                warm = (s0 + SB) <= W
                if k + 2 < NB:
                    xts[k + 2] = dma_block(k + 2)
                if k + 1 < NB:
                    znext = zp.tile([128, SB * L], f32, tag="z")
                    zs_blocks[k + 1] = znext
                z = zs_blocks.pop(k)

                ob = opool.tile([128, SB * L], f32, tag="ob")
                for s in range(SB):
                    gs = s0 + s
                    lo = s * L
                    zs = z[:, lo:lo + L]
                    o1A, o1B = o1[:, 0:HL], o1[:, HL:L]
                    u = sp.tile([128, L], f32, tag="u")
                    v = sp.tile([128, L], f32, tag="v")
                    vA, vB = v[:, 0:HL], v[:, HL:L]
                    # u is 2 steps off the critical path; emitted before the
                    # v's so it fills DVE's wait-for-tanh windows
                    nc.vector.scalar_tensor_tensor(u[:], o2, k_u, zs, op0=OP.mult, op1=OP.add)
                    if gs == W:  # chunk 0, t=0: o_{t-2} is carry col 1
                        fix_p0(nc.vector, u, c1, zs, k_u)
                    elif gs == W + 1:  # chunk 0, t=1: o_{t-2} is carry col 0
                        fix_p0(nc.vector, u, c0, zs, k_u)
                    # two half-lane chains: B's tanh overlaps A's handoff
                    nc.vector.scalar_tensor_tensor(vA, o1A, k_v, u[:, 0:HL], op0=OP.mult, op1=OP.add)
                    nc.vector.scalar_tensor_tensor(vB, o1B, k_v, u[:, HL:L], op0=OP.mult, op1=OP.add)
                    if gs == W:  # chunk 0, t=0: o_{t-1} is carry col 0
                        fix_p0(nc.vector, v, c0, u, k_v)
                    oslotA = ob[:, lo:lo + HL]
                    oslotB = ob[:, lo + HL:lo + L]
                    nc.scalar.activation(oslotA[:], vA[:], AF.Tanh, bias=0.0, scale=sc_act)
                    nc.scalar.activation(oslotB[:], vB[:], AF.Tanh, bias=0.0, scale=sc_act)
                    if k + 1 < NB and s % 4 == 0:
                        proj_sliver(k + 1, xts[k + 1], zs_blocks[k + 1], s, nsteps=4)
                    o2 = o1
                    o1 = ob[:, lo:lo + L]
                if not warm:
                    pending_out.append((ob, s0))
                # delay out-DMA issue ~8 blocks and put it on the SP ring:
                # SP executes its ring in program order, so every input read
                # ahead of the out in the stream gets the DMA fabric first --
                # the x stream is never throttled by output drains
                if len(pending_out) > 8:
                    dob, ds0 = pending_out.pop(0)
                    dob3 = dob[:].rearrange("p (s l) -> p s l", l=L)
                    nc.sync.dma_start(
                        out=bass.AP(out, (ds0 - W) * L, [[C * L, 128], [L, SB], [1, L]]),
                        in_=dob3[:])
            while pending_out:
                dob, ds0 = pending_out.pop(0)
                dob3 = dob[:].rearrange("p (s l) -> p s l", l=L)
                nc.sync.dma_start(
                    out=bass.AP(out, (ds0 - W) * L, [[C * L, 128], [L, SB], [1, L]]),
                    in_=dob3[:])
    nc.compile()
    return nc


def kernel(inputs, carry, weights):
    from concourse.bass_utils import run_bass_kernel_spmd

    key = np.asarray(weights, np.float32).tobytes()
    if key not in _cache:
        _cache[key] = _build(weights)
    nc = _cache[key]

    x = np.ascontiguousarray(np.asarray(inputs, np.float32))
    cr = np.ascontiguousarray(np.asarray(carry, np.float32))
    in_maps = []
    for c in range(NCORES):
        sl = slice(c * L, (c + 1) * L)
        in_maps.append({
            "inputs": np.ascontiguousarray(x[:, sl, :]),
            "carry": np.ascontiguousarray(cr[sl, :]),
        })
    res = run_bass_kernel_spmd(nc, in_maps, core_ids=list(range(NCORES)))
    outs = [r["out"] for r in res.results]
    return np.concatenate([o[:, :, None] for o in outs], axis=1)
